# revision 18
# baseline (speedup 1.0000x reference)
"""Trainium2 Bass kernel for nn_CapsuleLayer (dynamic-routing capsule layer).

Problem: x [B=64, N=2304, I=8], W [N, C=32, O=16, I=8]
  u_hat = einsum('ncoi,bni->bnco', W, x)
  3 routing iterations (softmax over N, weighted sum, squash, agreement)
  out = v [B, C, O, 1]

Strategy: shard N across 8 cores (288 n's each). Per core u_hat
(64*288*512 fp16 = 18.9MB) stays SBUF-resident. Routing iterations are
DVE-heavy: multiplies and reductions run as scalar_tensor_tensor ops in
the 4x DVE perf mode (all-fp16-SBUF, innermost stride 1). Softmax over N
and the batch-mean of the agreement couple the cores: one fused AllReduce
of [s_partial | z_partial] (266KB fp32) per iteration.

Layouts (per core):
  partition p = n2*64 + b   (n_local = n2*144 + nf, n2 in {0,1})
  u SBUF [128, nf=144, o=16, c=32] fp16, c innermost
  phase-1 matmul nf: lhsT = xblk[nf] [16, 128] (block-diag pair of x),
  rhs = wpair[nf] [16, (o,c)=512] -> PSUM [128, 512] -> cast fp16 to u
"""

import numpy as np

B, N, C, O, I = 64, 2304, 32, 16, 8
NCORES = 8
NS = N // NCORES        # 288 n per core
NF = NS // 2            # 144 pair-matmuls per core
MB = NF // 8            # 18 m-blocks of 8 pairs
CO = C * O              # 512
NITER = 3
NCH = 18                # chunks per pass (8 nf each)
CHNF = NF // NCH        # 8
NSL = NF // 2           # 72 weight slots, 2 pairs each (bases 0 and 32)

_cache = {}


# ---------------------------------------------------------------- device IR

def _build_nc():
    import concourse.bacc as bacc
    import concourse.bass as bass
    import concourse.mybir as mybir
    import concourse.tile as tile

    f16 = mybir.dt.float16
    f32 = mybir.dt.float32
    A = mybir.AluOpType
    AF = mybir.ActivationFunctionType

    nc = bacc.Bacc("TRN2", target_bir_lowering=False, debug=False,
                   num_devices=NCORES)

    xpack_d = nc.dram_tensor("xpack", [64, NSL, 128], f16, kind="ExternalInput")
    wpack_d = nc.dram_tensor("wpack", [NSL, 64, CO], f16, kind="ExternalInput")
    sel2_d = nc.dram_tensor("sel2", [128, 2], f16, kind="ExternalInput")
    sel2T_d = nc.dram_tensor("sel2T", [2, 128], f16, kind="ExternalInput")
    sel128_d = nc.dram_tensor("sel128", [64, 128], f16, kind="ExternalInput")
    vout_d = nc.dram_tensor("vout", [64, CO], f32, kind="ExternalOutput")

    groups = [list(range(NCORES))]

    with tile.TileContext(nc) as tc:
        import contextlib
        with contextlib.ExitStack() as ctx:
            const = ctx.enter_context(tc.tile_pool(name="const", bufs=1))
            dram = ctx.enter_context(
                tc.tile_pool(name="dram", bufs=1, space="DRAM"))
            upool = ctx.enter_context(tc.tile_pool(name="u", bufs=1))

            sel2 = const.tile([128, 2], f16)
            sel2T = const.tile([2, 128], f16)
            sel128 = const.tile([64, 128], f16)
            nc.sync.dma_start(sel2[:], sel2_d[:])
            nc.sync.dma_start(sel2T[:], sel2T_d[:])
            nc.sync.dma_start(sel128[:], sel128_d[:])

            u = upool.tile([128, O, NF, C], f16)

            # ---------------- phase 1: u_hat = x @ W (144 pair-matmuls)
            # slot m holds 2 pairs: nf=2m at partition base 0, nf=2m+1 at 32
            with tc.tile_pool(name="p1", bufs=1) as p1, \
                 tc.tile_pool(name="p1w", bufs=8) as p1w, \
                 tc.tile_pool(name="ps1", bufs=6, space="PSUM") as ps1:
                xres = p1.tile([64, NSL, 128], f16)
                for mm in range(8):
                    nc.sync.dma_start(xres[:, mm * 9:(mm + 1) * 9, :],
                                      xpack_d[:, mm * 9:(mm + 1) * 9, :])
                for m in range(NSL):
                    wsl = p1w.tile([64, CO], f16, tag="w", name="wsl")
                    nc.sync.dma_start(wsl[:], wpack_d[m])
                    for h in range(2):
                        nf = 2 * m + h
                        ps = ps1.tile([128, CO], f32, tag="ps", name="ps")
                        nc.tensor.matmul(
                            ps[:],
                            xres[32 * h:32 * h + 16, m, :],
                            wsl[32 * h:32 * h + 16, :],
                            start=True, stop=True)
                        src = ps[:].rearrange("p (o c) -> p o c", o=O)
                        dst = u[:, :, nf, :]
                        if nf % 6 == 5:
                            nc.scalar.activation(dst, src, AF.Copy)
                        else:
                            nc.vector.tensor_copy(dst, src)

            itp = ctx.enter_context(tc.tile_pool(name="itp", bufs=1))
            prodp = ctx.enter_context(tc.tile_pool(name="prod", bufs=1))
            pse = ctx.enter_context(
                tc.tile_pool(name="pse", bufs=2, space="PSUM"))
            psdb = ctx.enter_context(
                tc.tile_pool(name="psdb", bufs=2, space="PSUM"))
            psv = ctx.enter_context(
                tc.tile_pool(name="psv", bufs=1, space="PSUM"))

            # ---------------- persistent iteration tiles
            e_rep = itp.tile([128, NF * C], f16)       # exp(b) replicated
            a_acc = itp.tile([128, NF * C], f16)       # sum_o u*v
            b2 = itp.tile([2, NF * C], f16)            # logits [n2, (nf c)]
            z2 = itp.tile([2, C], f32)                 # local sum_nf e
            s_acc = itp.tile([128, O, C], f32)         # local s partials
            rz = itp.tile([64, C], f32)                # 1/z replicated
            zrep = itp.tile([64, 64], f32)
            zb = itp.tile([1, 64], f32)
            sg = itp.tile([64, O, C], f32)
            t0 = itp.tile([64, O, C], f32)
            t1 = itp.tile([64, O, C], f32)
            v16 = itp.tile([64, CO], f16)
            v_rep = itp.tile([128, CO], f16)

            NFC = NF * C  # 4608

            def prod_tile():
                # one o-slice of products, [128, (nf c)] fp16
                return prodp.tile([128, NFC], f16, tag="prod", name="pr")

            def fold_add(dst_ap, in0_ap, in1_ap):
                nc.vector.scalar_tensor_tensor(
                    dst_ap, in0_ap, 1.0, in1_ap, op0=A.mult, op1=A.add)

            def s_pass(it):
                """s_acc[p, o, c] = sum_nf (e*u) ; e==1 when it==0."""
                for o in range(O):
                    pr = prod_tile()
                    uo = u[:, o].rearrange("p nf c -> p (nf c)")  # [128, 4608]
                    if it == 0:
                        fold_add(pr[:, 0:2304], uo[:, 0:2304], uo[:, 2304:4608])
                    else:
                        nc.vector.scalar_tensor_tensor(
                            pr[:], uo, 1.0, e_rep[:],
                            op0=A.mult, op1=A.mult)
                        fold_add(pr[:, 0:2304], pr[:, 0:2304], pr[:, 2304:4608])
                    fold_add(pr[:, 0:1152], pr[:, 0:1152], pr[:, 1152:2304])
                    fold_add(pr[:, 0:576], pr[:, 0:576], pr[:, 576:1152])
                    fold_add(pr[:, 0:288], pr[:, 0:288], pr[:, 288:576])
                    # nf = 9 left (288 cols): 9 -> (8 -> 4 -> 2 -> 1) + 1
                    fold_add(pr[:, 0:128], pr[:, 0:128], pr[:, 128:256])
                    fold_add(pr[:, 0:64], pr[:, 0:64], pr[:, 64:128])
                    fold_add(pr[:, 0:32], pr[:, 0:32], pr[:, 32:64])
                    fold_add(pr[:, 0:32], pr[:, 0:32], pr[:, 256:288])
                    nc.vector.tensor_copy(s_acc[:, o, :], pr[:, 0:32])

            def allreduce(it):
                cc_in = dram.tile([65, 1024], f32, tag=f"ccin{it}", name=f"ccin{it}")
                cc_out = dram.tile([65, 1024], f32, tag=f"ccout{it}", name=f"ccout{it}")
                sf = s_acc[:].rearrange("p o c -> p (o c)")
                nc.sync.dma_start(cc_in[0:64, 0:512], sf[0:64])
                nc.sync.dma_start(cc_in[0:64, 512:1024], sf[64:128])
                if it > 0:
                    zrow = cc_in[64:65, 0:64].rearrange(
                        "a (b c) -> (a b) c", b=2)
                    nc.sync.dma_start(zrow, z2[:])
                else:
                    nc.vector.memset(z2[:], 0.0)
                    zrow = cc_in[64:65, 0:64].rearrange(
                        "a (b c) -> (a b) c", b=2)
                    nc.sync.dma_start(zrow, z2[:])
                nc.gpsimd.collective_compute(
                    "AllReduce", A.add, replica_groups=groups,
                    ins=[cc_in[:].opt()], outs=[cc_out[:].opt()])
                cc_s = prodp.tile([64, 2, 512], f32, tag="prod", name="cc_s")
                nc.sync.dma_start(cc_s[:], cc_out[0:64, :])
                if it > 0:
                    nc.sync.dma_start(zb[:], cc_out[64:65, 0:64])
                    nc.gpsimd.partition_broadcast(zrep[:], zb[:])
                    nc.vector.tensor_tensor(
                        rz[:], zrep[:, 0:32], zrep[:, 32:64], op=A.add)
                    nc.vector.reciprocal(rz[:], rz[:])
                else:
                    nc.vector.memset(rz[:], 1.0 / float(N))
                # sg = (half0 + half1) * rz  (global, normalized s)
                nc.vector.tensor_tensor(
                    sg[:], cc_s[:, 0].rearrange("p (o c) -> p o c", o=O),
                    cc_s[:, 1].rearrange("p (o c) -> p o c", o=O), op=A.add)
                rzb = rz[:].unsqueeze(1).broadcast_to((64, O, C))
                nc.vector.tensor_tensor(sg[:], sg[:], rzb, op=A.mult)

            def squash():
                """t1 = squash(sg) = sg*|sg| / (1+sg^2)"""
                nc.vector.tensor_tensor(t0[:], sg[:], sg[:], op=A.mult)
                nc.vector.tensor_scalar_add(t0[:], t0[:], 1.0)
                nc.vector.reciprocal(t0[:], t0[:])
                nc.vector.tensor_scalar_mul(t1[:], sg[:], -1.0)
                nc.vector.tensor_tensor(t1[:], sg[:], t1[:], op=A.max)
                nc.vector.tensor_tensor(t1[:], sg[:], t1[:], op=A.mult)
                nc.vector.tensor_tensor(t1[:], t1[:], t0[:], op=A.mult)

            def make_v_rep():
                nc.vector.tensor_copy(
                    v16[:], t1[:].rearrange("p o c -> p (o c)"))
                vp = psv.tile([128, CO], f32, tag="vps", name="vp")
                nc.tensor.matmul(vp[:], sel128[:], v16[:],
                                 start=True, stop=True)
                nc.vector.tensor_copy(v_rep[:], vp[:])

            def a_pass(it):
                """b2 += mean_b sum_o u*v ; writes e2, z2, e_rep."""
                for o in range(O):
                    uo = u[:, o]                       # [128, NF, C]
                    vo = v_rep[:, o * C:(o + 1) * C]   # [128, C]
                    vob = vo.unsqueeze(1).broadcast_to((128, NF, C))
                    aav = a_acc[:].rearrange("p (nf c) -> p nf c", nf=NF)
                    if o == 0:
                        nc.vector.scalar_tensor_tensor(
                            aav, uo, 1.0, vob, op0=A.mult, op1=A.mult)
                    else:
                        po = prod_tile()
                        pov = po[:].rearrange("p (nf c) -> p nf c", nf=NF)
                        nc.vector.scalar_tensor_tensor(
                            pov, uo, 1.0, vob, op0=A.mult, op1=A.mult)
                        fold_add(a_acc[:], po[:], a_acc[:])
                # db[n2, (nf c)] = sum_b a ; b2 += db/64
                for jj in range(9):
                    db = psdb.tile([2, 512], f32, tag="db", name="db")
                    nc.tensor.matmul(db[:], sel2[:],
                                     a_acc[:, jj * 512:(jj + 1) * 512],
                                     start=True, stop=True)
                    bsl = b2[:, jj * 512:(jj + 1) * 512]
                    if it == 0:
                        nc.vector.tensor_scalar_mul(bsl, db[:], 1.0 / B)
                    else:
                        nc.vector.scalar_tensor_tensor(
                            bsl, db[:], 1.0 / B, bsl, op0=A.mult, op1=A.add)
                e2 = prodp.tile([2, NF * C], f16, tag="prod", name="e2")
                nc.scalar.activation(e2[:], b2[:], AF.Exp)
                # z2[c] = sum_nf e2  (local partial)
                e2v = e2[:].rearrange("p (nf c) -> p c nf", c=C)
                nc.vector.tensor_reduce(z2[:], e2v, axis=mybir.AxisListType.X,
                                        op=A.add)
                # e_rep[p] = e2[p // 64]
                for jj in range(9):
                    ep = pse.tile([128, 512], f32, tag="eps", name="ep")
                    nc.tensor.matmul(ep[:], sel2T[:],
                                     e2[:, jj * 512:(jj + 1) * 512],
                                     start=True, stop=True)
                    nc.vector.tensor_copy(
                        e_rep[:, jj * 512:(jj + 1) * 512], ep[:])

            # ---------------- routing iterations
            for it in range(NITER):
                s_pass(it)
                allreduce(it)
                squash()
                if it < NITER - 1:
                    make_v_rep()
                    a_pass(it)
                else:
                    nc.sync.dma_start(
                        vout_d[:], t1[:].rearrange("p o c -> p (o c)"))

    nc.compile()
    return nc


# ---------------------------------------------------------------- host side

def _pack_inputs(x, W):
    """Build per-core input dicts (all fp16 packing done host-side)."""
    x = np.asarray(x, dtype=np.float32)
    W = np.asarray(W, dtype=np.float32)

    sel2 = np.zeros((128, 2), dtype=np.float16)
    sel2[0:64, 0] = 1.0
    sel2[64:128, 1] = 1.0
    sel2T = np.ascontiguousarray(sel2.T)
    sel128 = np.zeros((64, 128), dtype=np.float16)
    for p in range(128):
        sel128[p % 64, p] = 1.0

    in_maps = []
    for k in range(NCORES):
        xs = x[:, k * NS:(k + 1) * NS, :]        # [64, 288, 8]
        Ws = W[k * NS:(k + 1) * NS]              # [288, 32, 16, 8]

        # xpack[32*h + 8*n2 + i, m, 64*n2 + b] = xs[b, n2*144 + 2*m + h, i]
        # (rows 16-31 / 48-63 stay zero: matmul K tiles sit at bases 0/32)
        xpack = np.zeros((64, NSL, 128), dtype=np.float16)
        xv = xpack.reshape(2, 2, 2, 8, NSL, 2, 64)  # [h, pad, n2, i, m, n2c, b]
        A2 = xs.reshape(64, 2, NSL, 2, I)            # [b, n2, m, h, i]
        for n2 in range(2):
            # A2[:, n2] is [b, m, h, i] -> [h, i, m, b]
            xv[:, 0, n2, :, :, n2, :] = A2[:, n2].transpose(2, 3, 1, 0)
        # wpack[m, 32*h + 8*n2 + i, o*32 + c] = Ws[n2*144 + 2*m + h, c, o, i]
        wpack = np.zeros((NSL, 64, CO), dtype=np.float16)
        wv = wpack.reshape(NSL, 2, 2, 2, 8, O, C)   # [m, h, pad, n2, i, o, c]
        Wr = Ws.reshape(2, NSL, 2, C, O, I)          # [n2, m, h, c, o, i]
        for n2 in range(2):
            wv[:, :, 0, n2] = Wr[n2].transpose(0, 1, 4, 3, 2)  # [m, h, i, o, c]

        in_maps.append({
            "xpack": xpack,
            "wpack": wpack,
            "sel2": sel2,
            "sel2T": sel2T,
            "sel128": sel128,
        })
    return in_maps


def kernel(x, W):
    from concourse.bass_utils import run_bass_kernel_spmd

    if "nc" not in _cache:
        _cache["nc"] = _build_nc()
    nc = _cache["nc"]

    in_maps = _pack_inputs(x, W)
    res = run_bass_kernel_spmd(nc, in_maps, list(range(NCORES)))
    v = res.results[0]["vout"]                    # [64, (o c)] fp32
    v = v.reshape(B, O, C).transpose(0, 2, 1)[..., None]
    return np.ascontiguousarray(v.astype(np.float32))


# revision 23
# speedup vs baseline: 1.2867x; 1.2867x over previous
"""Trainium2 Bass kernel for nn_CapsuleLayer (dynamic-routing capsule layer).

Problem: x [B=64, N=2304, I=8], W [N, C=32, O=16, I=8]
  u_hat = einsum('ncoi,bni->bnco', W, x)
  3 routing iterations (softmax over N, weighted sum, squash, agreement)
  out = v [B, C, O, 1]

Strategy: shard N across 8 cores (288 n's each). Per core u_hat
(64*288*512 fp16 = 18.9MB) stays SBUF-resident. Routing iterations are
DVE-heavy: multiplies and reductions run as scalar_tensor_tensor ops in
the 4x DVE perf mode (all-fp16-SBUF, innermost stride 1). Softmax over N
and the batch-mean of the agreement couple the cores: one fused AllReduce
of [s_partial | z_partial] (266KB fp32) per iteration.

Layouts (per core):
  partition p = n2*64 + b   (n_local = n2*144 + nf, n2 in {0,1})
  u SBUF [128, nf=144, o=16, c=32] fp16, c innermost
  phase-1 matmul nf: lhsT = xblk[nf] [16, 128] (block-diag pair of x),
  rhs = wpair[nf] [16, (o,c)=512] -> PSUM [128, 512] -> cast fp16 to u
"""

import numpy as np

B, N, C, O, I = 64, 2304, 32, 16, 8
NCORES = 8
NS = N // NCORES        # 288 n per core
NF = NS // 2            # 144 pair-matmuls per core
MB = NF // 8            # 18 m-blocks of 8 pairs
CO = C * O              # 512
NITER = 3
NCH = 18                # chunks per pass (8 nf each)
CHNF = NF // NCH        # 8
NSL = NF // 2           # 72 weight slots, 2 pairs each (bases 0 and 32)

_cache = {}


# ---------------------------------------------------------------- device IR

def _build_nc():
    import concourse.bacc as bacc
    import concourse.bass as bass
    import concourse.mybir as mybir
    import concourse.tile as tile

    f16 = mybir.dt.float16
    f32 = mybir.dt.float32
    A = mybir.AluOpType
    AF = mybir.ActivationFunctionType

    nc = bacc.Bacc("TRN2", target_bir_lowering=False, debug=False,
                   num_devices=NCORES)

    xpack_d = nc.dram_tensor("xpack", [64, NSL, 128], f16, kind="ExternalInput")
    wpack_d = nc.dram_tensor("wpack", [NSL, 64, CO], f16, kind="ExternalInput")
    sel2_d = nc.dram_tensor("sel2", [128, 2], f16, kind="ExternalInput")
    sel2T_d = nc.dram_tensor("sel2T", [2, 128], f16, kind="ExternalInput")
    sel128_d = nc.dram_tensor("sel128", [64, 128], f16, kind="ExternalInput")
    vout_d = nc.dram_tensor("vout", [64, CO], f32, kind="ExternalOutput")

    groups = [list(range(NCORES))]

    with tile.TileContext(nc) as tc:
        import contextlib
        with contextlib.ExitStack() as ctx:
            const = ctx.enter_context(tc.tile_pool(name="const", bufs=1))
            dram = ctx.enter_context(
                tc.tile_pool(name="dram", bufs=1, space="DRAM"))
            upool = ctx.enter_context(tc.tile_pool(name="u", bufs=1))

            sel2 = const.tile([128, 2], f16)
            sel2T = const.tile([2, 128], f16)
            sel128 = const.tile([64, 128], f16)
            nc.sync.dma_start(sel2[:], sel2_d[:])
            nc.sync.dma_start(sel2T[:], sel2T_d[:])
            nc.sync.dma_start(sel128[:], sel128_d[:])

            u = upool.tile([128, O, NF, C], f16)

            # ---------------- phase 1: u_hat = x @ W (144 pair-matmuls)
            # slot m holds 2 pairs: nf=2m at partition base 0, nf=2m+1 at 32
            with tc.tile_pool(name="p1", bufs=1) as p1, \
                 tc.tile_pool(name="p1w", bufs=8) as p1w, \
                 tc.tile_pool(name="ps1", bufs=6, space="PSUM") as ps1:
                xres = p1.tile([64, NSL, 128], f16)
                for mm in range(8):
                    nc.sync.dma_start(xres[:, mm * 9:(mm + 1) * 9, :],
                                      xpack_d[:, mm * 9:(mm + 1) * 9, :])
                for m in range(NSL):
                    wsl = p1w.tile([64, CO], f16, tag="w", name="wsl")
                    nc.sync.dma_start(wsl[:], wpack_d[m])
                    for h in range(2):
                        nf = 2 * m + h
                        ps = ps1.tile([128, CO], f32, tag="ps", name="ps")
                        nc.tensor.matmul(
                            ps[:],
                            xres[32 * h:32 * h + 16, m, :],
                            wsl[32 * h:32 * h + 16, :],
                            start=True, stop=True)
                        src = ps[:].rearrange("p (o c) -> p o c", o=O)
                        dst = u[:, :, nf, :]
                        if nf % 3 == 2:
                            nc.scalar.activation(dst, src, AF.Copy)
                        else:
                            nc.vector.tensor_copy(dst, src)

            itp = ctx.enter_context(tc.tile_pool(name="itp", bufs=1))
            prodp = ctx.enter_context(tc.tile_pool(name="prod", bufs=1))
            pse = ctx.enter_context(
                tc.tile_pool(name="pse", bufs=2, space="PSUM"))
            psdb = ctx.enter_context(
                tc.tile_pool(name="psdb", bufs=2, space="PSUM"))
            psv = ctx.enter_context(
                tc.tile_pool(name="psv", bufs=1, space="PSUM"))

            # ---------------- persistent iteration tiles
            e_rep = itp.tile([128, NF * C], f16)       # exp(b) replicated
            a_acc = itp.tile([128, NF * C], f16)       # sum_o u*v
            b2 = itp.tile([2, NF * C], f16)            # logits [n2, (nf c)]
            z2 = itp.tile([2, C], f32)                 # local sum_nf e
            s_acc = itp.tile([128, O, C], f32)         # local s partials
            rz = itp.tile([64, C], f32)                # 1/z replicated
            zrep = itp.tile([64, 64], f32)
            zb = itp.tile([1, 64], f32)
            sg = itp.tile([64, O, C], f32)
            t0 = itp.tile([64, O, C], f32)
            t1 = itp.tile([64, O, C], f32)
            v16 = itp.tile([64, CO], f16)
            v_rep = itp.tile([128, CO], f16)

            NFC = NF * C  # 4608

            def prod_tile():
                # one o-slice of products, [128, (nf c)] fp16
                return prodp.tile([128, NFC], f16, tag="prod", name="pr")

            def tt(dst_ap, in0_ap, in1_ap, op=A.add):
                nc.vector.tensor_tensor(dst_ap, in0_ap, in1_ap, op=op)

            def s_pass(it):
                """s_acc[p, o, c] = sum_nf (e*u) ; e==1 when it==0.

                Folds overwrite their in0 range (out==in0 is full-rate on
                DVE; out==in1 is ~10x slower — never do that)."""
                for o in range(O):
                    pr = prod_tile()
                    uo = u[:, o].rearrange("p nf c -> p (nf c)")  # [128, 4608]
                    if it == 0:
                        tt(pr[:, 0:2304], uo[:, 0:2304], uo[:, 2304:4608])
                    else:
                        tt(pr[:], uo, e_rep[:], op=A.mult)
                        tt(pr[:, 0:2304], pr[:, 0:2304], pr[:, 2304:4608])
                    tt(pr[:, 0:1152], pr[:, 0:1152], pr[:, 1152:2304])
                    tt(pr[:, 0:576], pr[:, 0:576], pr[:, 576:1152])
                    tt(pr[:, 0:288], pr[:, 0:288], pr[:, 288:576])
                    # nf = 9 left: 8 -> 4 -> 2 -> 1, then add the 9th
                    tt(pr[:, 0:128], pr[:, 0:128], pr[:, 128:256])
                    tt(pr[:, 0:64], pr[:, 0:64], pr[:, 64:128])
                    tt(pr[:, 0:32], pr[:, 0:32], pr[:, 32:64])
                    tt(pr[:, 0:32], pr[:, 0:32], pr[:, 256:288])
                    nc.vector.tensor_copy(s_acc[:, o, :], pr[:, 0:32])

            def allreduce(it):
                cc_in = dram.tile([65, 1024], f32, tag=f"ccin{it}", name=f"ccin{it}")
                cc_out = dram.tile([65, 1024], f32, tag=f"ccout{it}", name=f"ccout{it}")
                sf = s_acc[:].rearrange("p o c -> p (o c)")
                nc.sync.dma_start(cc_in[0:64, 0:512], sf[0:64])
                nc.sync.dma_start(cc_in[0:64, 512:1024], sf[64:128])
                if it > 0:
                    zrow = cc_in[64:65, 0:64].rearrange(
                        "a (b c) -> (a b) c", b=2)
                    nc.sync.dma_start(zrow, z2[:])
                else:
                    nc.vector.memset(z2[:], 0.0)
                    zrow = cc_in[64:65, 0:64].rearrange(
                        "a (b c) -> (a b) c", b=2)
                    nc.sync.dma_start(zrow, z2[:])
                nc.gpsimd.collective_compute(
                    "AllReduce", A.add, replica_groups=groups,
                    ins=[cc_in[:].opt()], outs=[cc_out[:].opt()])
                cc_s = prodp.tile([64, 2, 512], f32, tag="prod", name="cc_s")
                nc.sync.dma_start(cc_s[:], cc_out[0:64, :])
                if it > 0:
                    nc.sync.dma_start(zb[:], cc_out[64:65, 0:64])
                    nc.gpsimd.partition_broadcast(zrep[:], zb[:])
                    nc.vector.tensor_tensor(
                        rz[:], zrep[:, 0:32], zrep[:, 32:64], op=A.add)
                    nc.vector.reciprocal(rz[:], rz[:])
                else:
                    nc.vector.memset(rz[:], 1.0 / float(N))
                # sg = (half0 + half1) * rz  (global, normalized s)
                nc.vector.tensor_tensor(
                    sg[:], cc_s[:, 0].rearrange("p (o c) -> p o c", o=O),
                    cc_s[:, 1].rearrange("p (o c) -> p o c", o=O), op=A.add)
                rzb = rz[:].unsqueeze(1).broadcast_to((64, O, C))
                nc.vector.tensor_tensor(sg[:], sg[:], rzb, op=A.mult)

            def squash():
                """t1 = squash(sg) = sg*|sg| / (1+sg^2)"""
                nc.vector.tensor_tensor(t0[:], sg[:], sg[:], op=A.mult)
                nc.vector.tensor_scalar_add(t0[:], t0[:], 1.0)
                nc.vector.reciprocal(t0[:], t0[:])
                nc.vector.tensor_scalar_mul(t1[:], sg[:], -1.0)
                nc.vector.tensor_tensor(t1[:], sg[:], t1[:], op=A.max)
                nc.vector.tensor_tensor(t1[:], sg[:], t1[:], op=A.mult)
                nc.vector.tensor_tensor(t1[:], t1[:], t0[:], op=A.mult)

            def make_v_rep():
                nc.vector.tensor_copy(
                    v16[:], t1[:].rearrange("p o c -> p (o c)"))
                vp = psv.tile([128, CO], f32, tag="vps", name="vp")
                nc.tensor.matmul(vp[:], sel128[:], v16[:],
                                 start=True, stop=True)
                nc.vector.tensor_copy(v_rep[:], vp[:])

            def a_pass(it):
                """b2 += mean_b sum_o u*v ; writes e2, z2, e_rep."""
                for o in range(O):
                    uo = u[:, o]                       # [128, NF, C]
                    vo = v_rep[:, o * C:(o + 1) * C]   # [128, C]
                    vob = vo.unsqueeze(1).broadcast_to((128, NF, C))
                    aav = a_acc[:].rearrange("p (nf c) -> p nf c", nf=NF)
                    if o == 0:
                        tt(aav, uo, vob, op=A.mult)
                    else:
                        po = prod_tile()
                        pov = po[:].rearrange("p (nf c) -> p nf c", nf=NF)
                        tt(pov, uo, vob, op=A.mult)
                        tt(a_acc[:], a_acc[:], po[:])
                # db[n2, (nf c)] = sum_b a/B  (1/B baked into sel2)
                for jj in range(9):
                    db = psdb.tile([2, 512], f32, tag="db", name="db")
                    nc.tensor.matmul(db[:], sel2[:],
                                     a_acc[:, jj * 512:(jj + 1) * 512],
                                     start=True, stop=True)
                    bsl = b2[:, jj * 512:(jj + 1) * 512]
                    if it == 0:
                        nc.vector.tensor_copy(bsl, db[:])
                    else:
                        tt(bsl, bsl, db[:])
                e2 = prodp.tile([2, NF * C], f16, tag="prod", name="e2")
                nc.scalar.activation(e2[:], b2[:], AF.Exp)
                # z2[c] = sum_nf e2  (local partial)
                e2v = e2[:].rearrange("p (nf c) -> p c nf", c=C)
                nc.vector.tensor_reduce(z2[:], e2v, axis=mybir.AxisListType.X,
                                        op=A.add)
                # e_rep[p] = e2[p // 64]
                for jj in range(9):
                    ep = pse.tile([128, 512], f32, tag="eps", name="ep")
                    nc.tensor.matmul(ep[:], sel2T[:],
                                     e2[:, jj * 512:(jj + 1) * 512],
                                     start=True, stop=True)
                    nc.vector.tensor_copy(
                        e_rep[:, jj * 512:(jj + 1) * 512], ep[:])

            # ---------------- routing iterations
            for it in range(NITER):
                s_pass(it)
                allreduce(it)
                squash()
                if it < NITER - 1:
                    make_v_rep()
                    a_pass(it)
                else:
                    nc.sync.dma_start(
                        vout_d[:], t1[:].rearrange("p o c -> p (o c)"))

    nc.compile()
    return nc


# ---------------------------------------------------------------- host side

def _pack_inputs(x, W):
    """Build per-core input dicts (all fp16 packing done host-side)."""
    x = np.asarray(x, dtype=np.float32)
    W = np.asarray(W, dtype=np.float32)

    # sel2 carries the 1/B scale of the batch-mean of a_ij
    sel2 = np.zeros((128, 2), dtype=np.float16)
    sel2[0:64, 0] = 1.0 / B
    sel2[64:128, 1] = 1.0 / B
    sel2T = np.zeros((2, 128), dtype=np.float16)
    sel2T[0, 0:64] = 1.0
    sel2T[1, 64:128] = 1.0
    sel128 = np.zeros((64, 128), dtype=np.float16)
    for p in range(128):
        sel128[p % 64, p] = 1.0

    in_maps = []
    for k in range(NCORES):
        xs = x[:, k * NS:(k + 1) * NS, :]        # [64, 288, 8]
        Ws = W[k * NS:(k + 1) * NS]              # [288, 32, 16, 8]

        # xpack[32*h + 8*n2 + i, m, 64*n2 + b] = xs[b, n2*144 + 2*m + h, i]
        # (rows 16-31 / 48-63 stay zero: matmul K tiles sit at bases 0/32)
        xpack = np.zeros((64, NSL, 128), dtype=np.float16)
        xv = xpack.reshape(2, 2, 2, 8, NSL, 2, 64)  # [h, pad, n2, i, m, n2c, b]
        A2 = xs.reshape(64, 2, NSL, 2, I)            # [b, n2, m, h, i]
        for n2 in range(2):
            # A2[:, n2] is [b, m, h, i] -> [h, i, m, b]
            xv[:, 0, n2, :, :, n2, :] = A2[:, n2].transpose(2, 3, 1, 0)
        # wpack[m, 32*h + 8*n2 + i, o*32 + c] = Ws[n2*144 + 2*m + h, c, o, i]
        wpack = np.zeros((NSL, 64, CO), dtype=np.float16)
        wv = wpack.reshape(NSL, 2, 2, 2, 8, O, C)   # [m, h, pad, n2, i, o, c]
        Wr = Ws.reshape(2, NSL, 2, C, O, I)          # [n2, m, h, c, o, i]
        for n2 in range(2):
            wv[:, :, 0, n2] = Wr[n2].transpose(0, 1, 4, 3, 2)  # [m, h, i, o, c]

        in_maps.append({
            "xpack": xpack,
            "wpack": wpack,
            "sel2": sel2,
            "sel2T": sel2T,
            "sel128": sel128,
        })
    return in_maps


def kernel(x, W):
    from concourse.bass_utils import run_bass_kernel_spmd

    if "nc" not in _cache:
        _cache["nc"] = _build_nc()
    nc = _cache["nc"]

    in_maps = _pack_inputs(x, W)
    res = run_bass_kernel_spmd(nc, in_maps, list(range(NCORES)))
    v = res.results[0]["vout"]                    # [64, (o c)] fp32
    v = v.reshape(B, O, C).transpose(0, 2, 1)[..., None]
    return np.ascontiguousarray(v.astype(np.float32))


# revision 26
# speedup vs baseline: 1.6306x; 1.2673x over previous
"""Trainium2 Bass kernel for nn_CapsuleLayer (dynamic-routing capsule layer).

Problem: x [B=64, N=2304, I=8], W [N, C=32, O=16, I=8]
  u_hat = einsum('ncoi,bni->bnco', W, x)
  3 routing iterations (softmax over N, weighted sum, squash, agreement)
  out = v [B, C, O, 1]

Strategy: shard N across 8 cores (288 n's each). Per core u_hat
(64*288*512 fp16 = 18.9MB) stays SBUF-resident. Routing iterations are
DVE-heavy: multiplies and reductions run as scalar_tensor_tensor ops in
the 4x DVE perf mode (all-fp16-SBUF, innermost stride 1). Softmax over N
and the batch-mean of the agreement couple the cores: one fused AllReduce
of [s_partial | z_partial] (266KB fp32) per iteration.

Layouts (per core):
  partition p = n2*64 + b   (n_local = n2*144 + nf, n2 in {0,1})
  u SBUF [128, nf=144, o=16, c=32] fp16, c innermost
  phase-1 matmul nf: lhsT = xblk[nf] [16, 128] (block-diag pair of x),
  rhs = wpair[nf] [16, (o,c)=512] -> PSUM [128, 512] -> cast fp16 to u
"""

import numpy as np

B, N, C, O, I = 64, 2304, 32, 16, 8
NCORES = 8
NS = N // NCORES        # 288 n per core
NF = NS // 2            # 144 pair-matmuls per core
MB = NF // 8            # 18 m-blocks of 8 pairs
CO = C * O              # 512
NITER = 3
NCH = 18                # chunks per pass (8 nf each)
CHNF = NF // NCH        # 8
NSL = NF // 2           # 72 weight slots, 2 pairs each (bases 0 and 32)

_cache = {}


# ---------------------------------------------------------------- device IR

def _build_nc():
    import concourse.bacc as bacc
    import concourse.bass as bass
    import concourse.mybir as mybir
    import concourse.tile as tile

    f16 = mybir.dt.float16
    f32 = mybir.dt.float32
    A = mybir.AluOpType
    AF = mybir.ActivationFunctionType

    nc = bacc.Bacc("TRN2", target_bir_lowering=False, debug=False,
                   num_devices=NCORES)

    xpack_d = nc.dram_tensor("xpack", [64, NSL, 128], f16, kind="ExternalInput")
    wpack_d = nc.dram_tensor("wpack", [NSL, 64, CO], f16, kind="ExternalInput")
    sel2_d = nc.dram_tensor("sel2", [128, 2], f16, kind="ExternalInput")
    sel2T_d = nc.dram_tensor("sel2T", [2, 128], f16, kind="ExternalInput")
    sel128_d = nc.dram_tensor("sel128", [64, 128], f16, kind="ExternalInput")
    vout_d = nc.dram_tensor("vout", [64, CO], f32, kind="ExternalOutput")

    groups = [list(range(NCORES))]

    with tile.TileContext(nc) as tc:
        import contextlib
        with contextlib.ExitStack() as ctx:
            const = ctx.enter_context(tc.tile_pool(name="const", bufs=1))
            dram = ctx.enter_context(
                tc.tile_pool(name="dram", bufs=1, space="DRAM"))
            upool = ctx.enter_context(tc.tile_pool(name="u", bufs=1))

            sel2 = const.tile([128, 2], f16)
            sel2T = const.tile([2, 128], f16)
            sel128 = const.tile([64, 128], f16)
            nc.sync.dma_start(sel2[:], sel2_d[:])
            nc.sync.dma_start(sel2T[:], sel2T_d[:])
            nc.sync.dma_start(sel128[:], sel128_d[:])

            u = upool.tile([128, O, NF, C], f16)

            # ---------------- phase 1: u_hat = x @ W (144 pair-matmuls)
            # slot m holds 2 pairs: nf=2m at partition base 0, nf=2m+1 at 32
            with tc.tile_pool(name="p1", bufs=1) as p1, \
                 tc.tile_pool(name="p1w", bufs=8) as p1w, \
                 tc.tile_pool(name="ps1", bufs=6, space="PSUM") as ps1:
                xres = p1.tile([64, NSL, 128], f16)
                for mm in range(8):
                    nc.sync.dma_start(xres[:, mm * 9:(mm + 1) * 9, :],
                                      xpack_d[:, mm * 9:(mm + 1) * 9, :])
                for m in range(NSL):
                    wsl = p1w.tile([64, CO], f16, tag="w", name="wsl")
                    nc.sync.dma_start(wsl[:], wpack_d[m])
                    for h in range(2):
                        nf = 2 * m + h
                        ps = ps1.tile([128, CO], f32, tag="ps", name="ps")
                        nc.tensor.matmul(
                            ps[:],
                            xres[32 * h:32 * h + 16, m, :],
                            wsl[32 * h:32 * h + 16, :],
                            start=True, stop=True)
                        src = ps[:].rearrange("p (o c) -> p o c", o=O)
                        dst = u[:, :, nf, :]
                        if nf % 3 == 2:
                            nc.scalar.activation(dst, src, AF.Copy)
                        else:
                            nc.vector.tensor_copy(dst, src)

            itp = ctx.enter_context(tc.tile_pool(name="itp", bufs=1))
            prodp = ctx.enter_context(tc.tile_pool(name="prod", bufs=1))
            pse = ctx.enter_context(
                tc.tile_pool(name="pse", bufs=2, space="PSUM"))
            psdb = ctx.enter_context(
                tc.tile_pool(name="psdb", bufs=2, space="PSUM"))
            psv = ctx.enter_context(
                tc.tile_pool(name="psv", bufs=1, space="PSUM"))

            # ---------------- persistent iteration tiles
            NFC = NF * C  # 4608

            e_rep = itp.tile([128, NFC], f16)          # exp(b) replicated
            b2 = itp.tile([2, NFC], f16)               # logits [n2, (nf c)]
            z2 = itp.tile([2, C], f32)                 # local sum_nf e
            s_acc = itp.tile([128, O, C], f32)         # local s partials
            rz = itp.tile([64, C], f32)                # 1/z replicated
            zrep = itp.tile([64, 64], f32)
            zb = itp.tile([1, 64], f32)
            sg = itp.tile([64, O, C], f32)
            t0 = itp.tile([64, O, C], f32)
            t1 = itp.tile([64, O, C], f32)
            v16 = itp.tile([64, CO], f16)
            v_rep = itp.tile([128, CO], f16)

            def scr9_tile(name):
                # [128, 4608] f16 slot shared by s_acc9 (s-pass) and
                # a_acc (a-pass) — disjoint lifetimes within an iteration
                return itp.tile([128, NFC], f16, tag="scr9", name=name)

            def pr2_tile():
                # o-pair product slot [128, 2, (nf c)] fp16
                return prodp.tile([128, 2, NFC], f16, tag="prod", name="pr2")

            def tt(dst_ap, in0_ap, in1_ap, op=A.add):
                nc.vector.tensor_tensor(dst_ap, in0_ap, in1_ap, op=op)

            def s_pass(it):
                """s_acc[p, o, c] = sum_nf (e*u) ; e==1 when it==0.

                o-pair at a time; folds overwrite their in0 range
                (out==in0 is full-rate on DVE; out==in1 is ~10x slower).
                Per-pair result (9 nf partials) lands in s_acc9; one
                batched strided tail then folds 9 -> 1 for all o."""
                s_acc9 = scr9_tile("s_acc9")
                s9 = s_acc9[:].rearrange("p (o g c) -> p o g c", o=O, g=9)
                for op_ in range(O // 2):
                    o = 2 * op_
                    pr = pr2_tile()
                    prf = pr[:].rearrange("p a x -> p (a x)")
                    u2 = u[:, o:o + 2].rearrange("p a nf c -> p (a nf c)")
                    if it == 0:
                        u2v = u[:, o:o + 2].rearrange("p a nf c -> p a (nf c)")
                        tt(pr[:, :, 0:2304], u2v[:, :, 0:2304],
                           u2v[:, :, 2304:4608])
                    else:
                        erb = e_rep[:].unsqueeze(1).broadcast_to((128, 2, NFC))
                        tt(pr[:].rearrange("p a x -> p a x"),
                           u[:, o:o + 2].rearrange("p a nf c -> p a (nf c)"),
                           erb, op=A.mult)
                        tt(pr[:, :, 0:2304], pr[:, :, 0:2304],
                           pr[:, :, 2304:4608])
                    tt(pr[:, :, 0:1152], pr[:, :, 0:1152], pr[:, :, 1152:2304])
                    tt(pr[:, :, 0:576], pr[:, :, 0:576], pr[:, :, 576:1152])
                    tt(s9[:, o:o + 2].rearrange("p a g c -> p a (g c)"),
                       pr[:, :, 0:288], pr[:, :, 288:576])
                # batched tail over all o: 9 -> (8 -> 4 -> 2 -> 1) + 1
                pr = pr2_tile()
                prf = pr[:].rearrange("p a x -> p (a x)")
                q1 = prf[:, 0:2048].rearrange("p (o x) -> p o x", o=O)
                tt(q1, s9[:, :, 0:4, :].rearrange("p o g c -> p o (g c)"),
                   s9[:, :, 4:8, :].rearrange("p o g c -> p o (g c)"))
                q2 = prf[:, 2048:3072].rearrange("p (o x) -> p o x", o=O)
                tt(q2, q1[:, :, 0:64], q1[:, :, 64:128])
                q3 = prf[:, 3072:3584].rearrange("p (o x) -> p o x", o=O)
                tt(q3, q2[:, :, 0:32], q2[:, :, 32:64])
                tt(s_acc[:].rearrange("p o c -> p o c"), q3,
                   s9[:, :, 8, :])

            def allreduce(it):
                cc_in = dram.tile([65, 1024], f32, tag=f"ccin{it}", name=f"ccin{it}")
                cc_out = dram.tile([65, 1024], f32, tag=f"ccout{it}", name=f"ccout{it}")
                sf = s_acc[:].rearrange("p o c -> p (o c)")
                nc.sync.dma_start(cc_in[0:64, 0:512], sf[0:64])
                nc.sync.dma_start(cc_in[0:64, 512:1024], sf[64:128])
                if it > 0:
                    zrow = cc_in[64:65, 0:64].rearrange(
                        "a (b c) -> (a b) c", b=2)
                    nc.sync.dma_start(zrow, z2[:])
                else:
                    nc.vector.memset(z2[:], 0.0)
                    zrow = cc_in[64:65, 0:64].rearrange(
                        "a (b c) -> (a b) c", b=2)
                    nc.sync.dma_start(zrow, z2[:])
                nc.gpsimd.collective_compute(
                    "AllReduce", A.add, replica_groups=groups,
                    ins=[cc_in[:].opt()], outs=[cc_out[:].opt()])
                cc_s = prodp.tile([64, 2, 512], f32, tag="prod", name="cc_s")
                nc.sync.dma_start(cc_s[:], cc_out[0:64, :])
                if it > 0:
                    nc.sync.dma_start(zb[:], cc_out[64:65, 0:64])
                    nc.gpsimd.partition_broadcast(zrep[:], zb[:])
                    nc.vector.tensor_tensor(
                        rz[:], zrep[:, 0:32], zrep[:, 32:64], op=A.add)
                    nc.vector.reciprocal(rz[:], rz[:])
                else:
                    nc.vector.memset(rz[:], 1.0 / float(N))
                # sg = (half0 + half1) * rz  (global, normalized s)
                nc.vector.tensor_tensor(
                    sg[:], cc_s[:, 0].rearrange("p (o c) -> p o c", o=O),
                    cc_s[:, 1].rearrange("p (o c) -> p o c", o=O), op=A.add)
                rzb = rz[:].unsqueeze(1).broadcast_to((64, O, C))
                nc.vector.tensor_tensor(sg[:], sg[:], rzb, op=A.mult)

            def squash():
                """t1 = squash(sg) = sg*|sg| / (1+sg^2)"""
                nc.vector.tensor_tensor(t0[:], sg[:], sg[:], op=A.mult)
                nc.vector.tensor_scalar_add(t0[:], t0[:], 1.0)
                nc.vector.reciprocal(t0[:], t0[:])
                nc.vector.tensor_scalar_mul(t1[:], sg[:], -1.0)
                nc.vector.tensor_tensor(t1[:], sg[:], t1[:], op=A.max)
                nc.vector.tensor_tensor(t1[:], sg[:], t1[:], op=A.mult)
                nc.vector.tensor_tensor(t1[:], t1[:], t0[:], op=A.mult)

            def make_v_rep():
                nc.vector.tensor_copy(
                    v16[:], t1[:].rearrange("p o c -> p (o c)"))
                vp = psv.tile([128, CO], f32, tag="vps", name="vp")
                nc.tensor.matmul(vp[:], sel128[:], v16[:],
                                 start=True, stop=True)
                nc.vector.tensor_copy(v_rep[:], vp[:])

            def a_pass(it):
                """b2 += mean_b sum_o u*v ; writes e2, z2, e_rep."""
                a_acc = scr9_tile("a_acc")
                for o in range(O):
                    uo = u[:, o]                       # [128, NF, C]
                    vo = v_rep[:, o * C:(o + 1) * C]   # [128, C]
                    vob = vo.unsqueeze(1).broadcast_to((128, NF, C))
                    aav = a_acc[:].rearrange("p (nf c) -> p nf c", nf=NF)
                    if o == 0:
                        tt(aav, uo, vob, op=A.mult)
                    else:
                        po = prodp.tile([128, NFC], f16, tag="prod", name="po")
                        pov = po[:].rearrange("p (nf c) -> p nf c", nf=NF)
                        tt(pov, uo, vob, op=A.mult)
                        tt(a_acc[:], a_acc[:], po[:])
                # db[n2, (nf c)] = sum_b a/B  (1/B baked into sel2)
                for jj in range(9):
                    db = psdb.tile([2, 512], f32, tag="db", name="db")
                    nc.tensor.matmul(db[:], sel2[:],
                                     a_acc[:, jj * 512:(jj + 1) * 512],
                                     start=True, stop=True)
                    bsl = b2[:, jj * 512:(jj + 1) * 512]
                    if it == 0:
                        nc.vector.tensor_copy(bsl, db[:])
                    else:
                        tt(bsl, bsl, db[:])
                e2 = prodp.tile([2, NF * C], f16, tag="prod", name="e2")
                nc.scalar.activation(e2[:], b2[:], AF.Exp)
                # z2[c] = sum_nf e2  (local partial)
                e2v = e2[:].rearrange("p (nf c) -> p c nf", c=C)
                nc.vector.tensor_reduce(z2[:], e2v, axis=mybir.AxisListType.X,
                                        op=A.add)
                # e_rep[p] = e2[p // 64]
                for jj in range(9):
                    ep = pse.tile([128, 512], f32, tag="eps", name="ep")
                    nc.tensor.matmul(ep[:], sel2T[:],
                                     e2[:, jj * 512:(jj + 1) * 512],
                                     start=True, stop=True)
                    nc.vector.tensor_copy(
                        e_rep[:, jj * 512:(jj + 1) * 512], ep[:])

            # ---------------- routing iterations
            for it in range(NITER):
                s_pass(it)
                allreduce(it)
                squash()
                if it < NITER - 1:
                    make_v_rep()
                    a_pass(it)
                else:
                    nc.sync.dma_start(
                        vout_d[:], t1[:].rearrange("p o c -> p (o c)"))

    nc.compile()
    return nc


# ---------------------------------------------------------------- host side

def _pack_inputs(x, W):
    """Build per-core input dicts (all fp16 packing done host-side)."""
    x = np.asarray(x, dtype=np.float32)
    W = np.asarray(W, dtype=np.float32)

    # sel2 carries the 1/B scale of the batch-mean of a_ij
    sel2 = np.zeros((128, 2), dtype=np.float16)
    sel2[0:64, 0] = 1.0 / B
    sel2[64:128, 1] = 1.0 / B
    sel2T = np.zeros((2, 128), dtype=np.float16)
    sel2T[0, 0:64] = 1.0
    sel2T[1, 64:128] = 1.0
    sel128 = np.zeros((64, 128), dtype=np.float16)
    for p in range(128):
        sel128[p % 64, p] = 1.0

    in_maps = []
    for k in range(NCORES):
        xs = x[:, k * NS:(k + 1) * NS, :]        # [64, 288, 8]
        Ws = W[k * NS:(k + 1) * NS]              # [288, 32, 16, 8]

        # xpack[32*h + 8*n2 + i, m, 64*n2 + b] = xs[b, n2*144 + 2*m + h, i]
        # (rows 16-31 / 48-63 stay zero: matmul K tiles sit at bases 0/32)
        xpack = np.zeros((64, NSL, 128), dtype=np.float16)
        xv = xpack.reshape(2, 2, 2, 8, NSL, 2, 64)  # [h, pad, n2, i, m, n2c, b]
        A2 = xs.reshape(64, 2, NSL, 2, I)            # [b, n2, m, h, i]
        for n2 in range(2):
            # A2[:, n2] is [b, m, h, i] -> [h, i, m, b]
            xv[:, 0, n2, :, :, n2, :] = A2[:, n2].transpose(2, 3, 1, 0)
        # wpack[m, 32*h + 8*n2 + i, o*32 + c] = Ws[n2*144 + 2*m + h, c, o, i]
        wpack = np.zeros((NSL, 64, CO), dtype=np.float16)
        wv = wpack.reshape(NSL, 2, 2, 2, 8, O, C)   # [m, h, pad, n2, i, o, c]
        Wr = Ws.reshape(2, NSL, 2, C, O, I)          # [n2, m, h, c, o, i]
        for n2 in range(2):
            wv[:, :, 0, n2] = Wr[n2].transpose(0, 1, 4, 3, 2)  # [m, h, i, o, c]

        in_maps.append({
            "xpack": xpack,
            "wpack": wpack,
            "sel2": sel2,
            "sel2T": sel2T,
            "sel128": sel128,
        })
    return in_maps


def kernel(x, W):
    from concourse.bass_utils import run_bass_kernel_spmd

    if "nc" not in _cache:
        _cache["nc"] = _build_nc()
    nc = _cache["nc"]

    in_maps = _pack_inputs(x, W)
    res = run_bass_kernel_spmd(nc, in_maps, list(range(NCORES)))
    v = res.results[0]["vout"]                    # [64, (o c)] fp32
    v = v.reshape(B, O, C).transpose(0, 2, 1)[..., None]
    return np.ascontiguousarray(v.astype(np.float32))


# revision 31
# speedup vs baseline: 1.7268x; 1.0590x over previous
"""Trainium2 Bass kernel for nn_CapsuleLayer (dynamic-routing capsule layer).

Problem: x [B=64, N=2304, I=8], W [N, C=32, O=16, I=8]
  u_hat = einsum('ncoi,bni->bnco', W, x)
  3 routing iterations (softmax over N, weighted sum, squash, agreement)
  out = v [B, C, O, 1]

Strategy: shard N across 8 cores (288 n's each). Per core u_hat
(64*288*512 fp16 = 18.9MB) stays SBUF-resident. Routing iterations are
DVE-heavy: multiplies and reductions run as scalar_tensor_tensor ops in
the 4x DVE perf mode (all-fp16-SBUF, innermost stride 1). Softmax over N
and the batch-mean of the agreement couple the cores: one fused AllReduce
of [s_partial | z_partial] (266KB fp32) per iteration.

Layouts (per core):
  partition p = n2*64 + b   (n_local = n2*144 + nf, n2 in {0,1})
  u SBUF [128, nf=144, o=16, c=32] fp16, c innermost
  phase-1 matmul nf: lhsT = xblk[nf] [16, 128] (block-diag pair of x),
  rhs = wpair[nf] [16, (o,c)=512] -> PSUM [128, 512] -> cast fp16 to u
"""

import numpy as np

B, N, C, O, I = 64, 2304, 32, 16, 8
NCORES = 8
NS = N // NCORES        # 288 n per core
NF = NS // 2            # 144 pair-matmuls per core
MB = NF // 8            # 18 m-blocks of 8 pairs
CO = C * O              # 512
NITER = 3
NCH = 18                # chunks per pass (8 nf each)
CHNF = NF // NCH        # 8
NSL = NF // 2           # 72 weight slots, 2 pairs each (bases 0 and 32)

_cache = {}


# ---------------------------------------------------------------- device IR

def _build_nc():
    import concourse.bacc as bacc
    import concourse.bass as bass
    import concourse.mybir as mybir
    import concourse.tile as tile

    f16 = mybir.dt.float16
    f32 = mybir.dt.float32
    A = mybir.AluOpType
    AF = mybir.ActivationFunctionType

    nc = bacc.Bacc("TRN2", target_bir_lowering=False, debug=False,
                   num_devices=NCORES)

    xpack_d = nc.dram_tensor("xpack", [64, NSL, 128], f16, kind="ExternalInput")
    wpack_d = nc.dram_tensor("wpack", [NSL, 64, CO], f16, kind="ExternalInput")
    sel2_d = nc.dram_tensor("sel2", [128, 2], f16, kind="ExternalInput")
    sel2T_d = nc.dram_tensor("sel2T", [2, 128], f16, kind="ExternalInput")
    sel128_d = nc.dram_tensor("sel128", [64, 128], f16, kind="ExternalInput")
    ident_d = nc.dram_tensor("ident", [128, 128], f16, kind="ExternalInput")
    vout_d = nc.dram_tensor("vout", [64, CO], f32, kind="ExternalOutput")

    groups = [list(range(NCORES))]

    with tile.TileContext(nc) as tc:
        import contextlib
        with contextlib.ExitStack() as ctx:
            const = ctx.enter_context(tc.tile_pool(name="const", bufs=1))
            dram = ctx.enter_context(
                tc.tile_pool(name="dram", bufs=1, space="DRAM"))
            upool = ctx.enter_context(tc.tile_pool(name="u", bufs=1))

            sel2 = const.tile([128, 2], f16)
            sel2T = const.tile([2, 128], f16)
            sel128 = const.tile([64, 128], f16)
            ident = const.tile([128, 128], f16)
            nc.sync.dma_start(sel2[:], sel2_d[:])
            nc.sync.dma_start(sel2T[:], sel2T_d[:])
            nc.sync.dma_start(sel128[:], sel128_d[:])
            nc.sync.dma_start(ident[:], ident_d[:])

            u = upool.tile([128, O, NF, C], f16)

            # ---------------- phase 1: u_hat = x @ W (144 pair-matmuls)
            # slot m holds 2 pairs: nf=2m at partition base 0, nf=2m+1 at 32
            with tc.tile_pool(name="p1", bufs=1) as p1, \
                 tc.tile_pool(name="p1w", bufs=8) as p1w, \
                 tc.tile_pool(name="ps1", bufs=6, space="PSUM") as ps1:
                xres = p1.tile([64, NSL, 128], f16)
                for mm in range(8):
                    nc.sync.dma_start(xres[:, mm * 9:(mm + 1) * 9, :],
                                      xpack_d[:, mm * 9:(mm + 1) * 9, :])
                for m in range(NSL):
                    wsl = p1w.tile([64, CO], f16, tag="w", name="wsl")
                    nc.sync.dma_start(wsl[:], wpack_d[m])
                    for h in range(2):
                        nf = 2 * m + h
                        ps = ps1.tile([128, CO], f32, tag="ps", name="ps")
                        nc.tensor.matmul(
                            ps[:],
                            xres[32 * h:32 * h + 16, m, :],
                            wsl[32 * h:32 * h + 16, :],
                            start=True, stop=True)
                        src = ps[:].rearrange("p (o c) -> p o c", o=O)
                        dst = u[:, :, nf, :]
                        if nf % 3 == 2:
                            nc.scalar.activation(dst, src, AF.Copy)
                        else:
                            nc.vector.tensor_copy(dst, src)

            itp = ctx.enter_context(tc.tile_pool(name="itp", bufs=1))
            prodp = ctx.enter_context(tc.tile_pool(name="prod", bufs=1))
            pse = ctx.enter_context(
                tc.tile_pool(name="pse", bufs=1, space="PSUM"))
            psdb = ctx.enter_context(
                tc.tile_pool(name="psdb", bufs=1, space="PSUM"))
            psv = ctx.enter_context(
                tc.tile_pool(name="psv", bufs=1, space="PSUM"))
            psa = ctx.enter_context(
                tc.tile_pool(name="psa", bufs=1, space="PSUM"))

            # ---------------- persistent iteration tiles
            NFC = NF * C  # 4608

            e_rep = itp.tile([128, NFC], f16)          # exp(b) replicated
            b2 = itp.tile([2, NFC], f16)               # logits [n2, (nf c)]
            z2 = itp.tile([2, C], f32)                 # local sum_nf e
            s_acc = itp.tile([128, O, C], f16)         # local s partials
            z2f = itp.tile([2, C], f16)
            rz = itp.tile([64, C], f32)                # 1/z replicated
            zrep = itp.tile([64, 64], f16)
            zb = itp.tile([1, 64], f16)
            sg = itp.tile([64, O, C], f32)
            t0 = itp.tile([64, O, C], f32)
            t1 = itp.tile([64, O, C], f32)
            v16 = itp.tile([64, CO], f16)
            v_rep = itp.tile([128, CO], f16)

            def scr9_tile(name):
                # [128, 4608] f16 slot shared by s_acc9 (s-pass) and
                # a_acc (a-pass) — disjoint lifetimes within an iteration
                return itp.tile([128, NFC], f16, tag="scr9", name=name)

            def pr2_tile():
                # o-pair product slot [128, 2, (nf c)] fp16
                return prodp.tile([128, 2, NFC], f16, tag="prod", name="pr2")

            def tt(dst_ap, in0_ap, in1_ap, op=A.add):
                nc.vector.tensor_tensor(dst_ap, in0_ap, in1_ap, op=op)

            def s_pass(it):
                """s_acc[p, o, c] = sum_nf (e*u) ; e==1 when it==0.

                o-pair at a time; folds overwrite their in0 range
                (out==in0 is full-rate on DVE; out==in1 is ~10x slower).
                Per-pair result (9 nf partials) lands in s_acc9; one
                batched strided tail then folds 9 -> 1 for all o."""
                s_acc9 = scr9_tile("s_acc9")
                s9 = s_acc9[:].rearrange("p (o g c) -> p o g c", o=O, g=9)
                for op_ in range(O // 2):
                    o = 2 * op_
                    pr = pr2_tile()
                    prf = pr[:].rearrange("p a x -> p (a x)")
                    u2 = u[:, o:o + 2].rearrange("p a nf c -> p (a nf c)")
                    if it == 0:
                        u2v = u[:, o:o + 2].rearrange("p a nf c -> p a (nf c)")
                        tt(pr[:, :, 0:2304], u2v[:, :, 0:2304],
                           u2v[:, :, 2304:4608])
                    else:
                        erb = e_rep[:].unsqueeze(1).broadcast_to((128, 2, NFC))
                        tt(pr[:].rearrange("p a x -> p a x"),
                           u[:, o:o + 2].rearrange("p a nf c -> p a (nf c)"),
                           erb, op=A.mult)
                        tt(pr[:, :, 0:2304], pr[:, :, 0:2304],
                           pr[:, :, 2304:4608])
                    tt(pr[:, :, 0:1152], pr[:, :, 0:1152], pr[:, :, 1152:2304])
                    tt(pr[:, :, 0:576], pr[:, :, 0:576], pr[:, :, 576:1152])
                    tt(s9[:, o:o + 2].rearrange("p a g c -> p a (g c)"),
                       pr[:, :, 0:288], pr[:, :, 288:576])
                # batched tail over all o: 9 -> (8 -> 4 -> 2 -> 1) + 1
                pr = pr2_tile()
                prf = pr[:].rearrange("p a x -> p (a x)")
                q1 = prf[:, 0:2048].rearrange("p (o x) -> p o x", o=O)
                tt(q1, s9[:, :, 0:4, :].rearrange("p o g c -> p o (g c)"),
                   s9[:, :, 4:8, :].rearrange("p o g c -> p o (g c)"))
                q2 = prf[:, 2048:3072].rearrange("p (o x) -> p o x", o=O)
                tt(q2, q1[:, :, 0:64], q1[:, :, 64:128])
                q3 = prf[:, 3072:3584].rearrange("p (o x) -> p o x", o=O)
                tt(q3, q2[:, :, 0:32], q2[:, :, 32:64])
                tt(s_acc[:].rearrange("p o c -> p o c"), q3,
                   s9[:, :, 8, :])

            def allreduce(it):
                cc_in = dram.tile([65, 1024], f16, tag=f"ccin{it}", name=f"ccin{it}")
                cc_out = dram.tile([65, 1024], f16, tag=f"ccout{it}", name=f"ccout{it}")
                sf = s_acc[:].rearrange("p o c -> p (o c)")
                nc.sync.dma_start(cc_in[0:64, 0:512], sf[0:64])
                nc.sync.dma_start(cc_in[0:64, 512:1024], sf[64:128])
                zrow = cc_in[64:65, 0:64].rearrange("a (b c) -> (a b) c", b=2)
                if it > 0:
                    nc.vector.tensor_copy(z2f[:], z2[:])
                else:
                    nc.vector.memset(z2f[:], 0.0)
                nc.sync.dma_start(zrow, z2f[:])
                nc.gpsimd.collective_compute(
                    "AllReduce", A.add, replica_groups=groups,
                    ins=[cc_in[:].opt()], outs=[cc_out[:].opt()])
                cc_s = prodp.tile([64, 2, 512], f16, tag="prod", name="cc_s")
                nc.sync.dma_start(cc_s[:], cc_out[0:64, :])
                if it > 0:
                    nc.sync.dma_start(zb[:], cc_out[64:65, 0:64])
                    nc.gpsimd.partition_broadcast(zrep[:], zb[:])
                    nc.vector.tensor_tensor(
                        rz[:], zrep[:, 0:32], zrep[:, 32:64], op=A.add)
                    nc.vector.reciprocal(rz[:], rz[:])
                else:
                    nc.vector.memset(rz[:], 1.0 / float(N))
                # sg = (half0 + half1) * rz  (global, normalized s)
                nc.vector.tensor_tensor(
                    sg[:], cc_s[:, 0].rearrange("p (o c) -> p o c", o=O),
                    cc_s[:, 1].rearrange("p (o c) -> p o c", o=O), op=A.add)
                rzb = rz[:].unsqueeze(1).broadcast_to((64, O, C))
                nc.vector.tensor_tensor(sg[:], sg[:], rzb, op=A.mult)

            def squash():
                """t1 = squash(sg) = sg*|sg| / (1+sg^2)"""
                nc.vector.tensor_tensor(t0[:], sg[:], sg[:], op=A.mult)
                nc.vector.tensor_scalar_add(t0[:], t0[:], 1.0)
                nc.vector.reciprocal(t0[:], t0[:])
                nc.vector.tensor_scalar_mul(t1[:], sg[:], -1.0)
                nc.vector.tensor_tensor(t1[:], sg[:], t1[:], op=A.max)
                nc.vector.tensor_tensor(t1[:], sg[:], t1[:], op=A.mult)
                nc.vector.tensor_tensor(t1[:], t1[:], t0[:], op=A.mult)

            def make_v_rep():
                nc.vector.tensor_copy(
                    v16[:], t1[:].rearrange("p o c -> p (o c)"))
                vp = psv.tile([128, CO], f32, tag="vps", name="vp")
                nc.tensor.matmul(vp[:], sel128[:], v16[:],
                                 start=True, stop=True)
                nc.vector.tensor_copy(v_rep[:], vp[:])

            PEC = 2560  # columns summed over o on PE (5 psum banks)

            def a_pass(it):
                """b2 += mean_b sum_o u*v ; writes e2, z2, e_rep.

                Hybrid o-sum: PE identity-matmuls accumulate product
                columns [0, PEC) in PSUM while DVE chain-adds the rest."""
                a_acc = scr9_tile("a_acc")
                ps = psa.tile([128, PEC], f32, name="psacc")
                pr = pr2_tile()   # two [128, NFC] halves ping-pong as po
                for o in range(O):
                    uo = u[:, o]                       # [128, NF, C]
                    vo = v_rep[:, o * C:(o + 1) * C]   # [128, C]
                    vob = vo.unsqueeze(1).broadcast_to((128, NF, C))
                    if o == 0:
                        po = a_acc[:]                  # first product
                    else:
                        po = pr[:, o % 2, :]
                    tt(po.rearrange("p (nf c) -> p nf c", nf=NF), uo, vob,
                       op=A.mult)
                    for j in range(PEC // 512):
                        nc.tensor.matmul(
                            ps[:, j * 512:(j + 1) * 512], ident[:],
                            po[:, j * 512:(j + 1) * 512],
                            start=(o == 0), stop=(o == O - 1))
                    if o > 0:
                        tt(a_acc[:, PEC:NFC], a_acc[:, PEC:NFC],
                           po[:, PEC:NFC])
                nc.vector.tensor_copy(a_acc[:, 0:PEC], ps[:])
                # db[n2, (nf c)] = sum_b a/B  (1/B baked into sel2)
                for jj in range(9):
                    db = psdb.tile([2, 512], f32, tag="db", name="db")
                    nc.tensor.matmul(db[:], sel2[:],
                                     a_acc[:, jj * 512:(jj + 1) * 512],
                                     start=True, stop=True)
                    bsl = b2[:, jj * 512:(jj + 1) * 512]
                    if it == 0:
                        nc.vector.tensor_copy(bsl, db[:])
                    else:
                        tt(bsl, bsl, db[:])
                e2 = prodp.tile([2, NF * C], f16, tag="prod", name="e2")
                nc.scalar.activation(e2[:], b2[:], AF.Exp)
                # z2[c] = sum_nf e2  (local partial)
                e2v = e2[:].rearrange("p (nf c) -> p c nf", c=C)
                nc.vector.tensor_reduce(z2[:], e2v, axis=mybir.AxisListType.X,
                                        op=A.add)
                # e_rep[p] = e2[p // 64]
                for jj in range(9):
                    ep = pse.tile([128, 512], f32, tag="eps", name="ep")
                    nc.tensor.matmul(ep[:], sel2T[:],
                                     e2[:, jj * 512:(jj + 1) * 512],
                                     start=True, stop=True)
                    nc.vector.tensor_copy(
                        e_rep[:, jj * 512:(jj + 1) * 512], ep[:])

            # ---------------- routing iterations
            for it in range(NITER):
                s_pass(it)
                allreduce(it)
                squash()
                if it < NITER - 1:
                    make_v_rep()
                    a_pass(it)
                else:
                    nc.sync.dma_start(
                        vout_d[:], t1[:].rearrange("p o c -> p (o c)"))

    nc.compile()
    return nc


# ---------------------------------------------------------------- host side

def _pack_inputs(x, W):
    """Build per-core input dicts (all fp16 packing done host-side)."""
    x = np.asarray(x, dtype=np.float32)
    W = np.asarray(W, dtype=np.float32)

    # sel2 carries the 1/B scale of the batch-mean of a_ij
    sel2 = np.zeros((128, 2), dtype=np.float16)
    sel2[0:64, 0] = 1.0 / B
    sel2[64:128, 1] = 1.0 / B
    sel2T = np.zeros((2, 128), dtype=np.float16)
    sel2T[0, 0:64] = 1.0
    sel2T[1, 64:128] = 1.0
    sel128 = np.zeros((64, 128), dtype=np.float16)
    for p in range(128):
        sel128[p % 64, p] = 1.0
    ident = np.eye(128, dtype=np.float16)

    in_maps = []
    for k in range(NCORES):
        xs = x[:, k * NS:(k + 1) * NS, :]        # [64, 288, 8]
        Ws = W[k * NS:(k + 1) * NS]              # [288, 32, 16, 8]

        # xpack[32*h + 8*n2 + i, m, 64*n2 + b] = xs[b, n2*144 + 2*m + h, i]
        # (rows 16-31 / 48-63 stay zero: matmul K tiles sit at bases 0/32)
        xpack = np.zeros((64, NSL, 128), dtype=np.float16)
        xv = xpack.reshape(2, 2, 2, 8, NSL, 2, 64)  # [h, pad, n2, i, m, n2c, b]
        A2 = xs.reshape(64, 2, NSL, 2, I)            # [b, n2, m, h, i]
        for n2 in range(2):
            # A2[:, n2] is [b, m, h, i] -> [h, i, m, b]
            xv[:, 0, n2, :, :, n2, :] = A2[:, n2].transpose(2, 3, 1, 0)
        # wpack[m, 32*h + 8*n2 + i, o*32 + c] = Ws[n2*144 + 2*m + h, c, o, i]
        wpack = np.zeros((NSL, 64, CO), dtype=np.float16)
        wv = wpack.reshape(NSL, 2, 2, 2, 8, O, C)   # [m, h, pad, n2, i, o, c]
        Wr = Ws.reshape(2, NSL, 2, C, O, I)          # [n2, m, h, c, o, i]
        for n2 in range(2):
            wv[:, :, 0, n2] = Wr[n2].transpose(0, 1, 4, 3, 2)  # [m, h, i, o, c]

        in_maps.append({
            "xpack": xpack,
            "wpack": wpack,
            "sel2": sel2,
            "sel2T": sel2T,
            "sel128": sel128,
            "ident": ident,
        })
    return in_maps


def kernel(x, W):
    from concourse.bass_utils import run_bass_kernel_spmd

    if "nc" not in _cache:
        _cache["nc"] = _build_nc()
    nc = _cache["nc"]

    in_maps = _pack_inputs(x, W)
    res = run_bass_kernel_spmd(nc, in_maps, list(range(NCORES)))
    v = res.results[0]["vout"]                    # [64, (o c)] fp32
    v = v.reshape(B, O, C).transpose(0, 2, 1)[..., None]
    return np.ascontiguousarray(v.astype(np.float32))
